# revision 23
# baseline (speedup 1.0000x reference)
"""Trainium2 Bass kernel for ChunkTriangleAttentionStartingNode.

Computation (B=1, N=384, D=128, h=4, c=32):
  Z = LayerNorm(Z_raw) * ln_w + ln_b                     (over d_pair)
  bias[h,q,k]   = (Z @ W_b)[q,k,h]        (triangle bias, row-indexed by q)
  q,k,v         = split(Z @ W_qkv)        per pair-row i, heads h, dim c
  logits[i,h,q,k] = q.k / sqrt(c) + mask_bias[i,k] + bias[h,q,k]
  out = Z_raw + (sigmoid(Z@W_gate + gb) * softmax(logits) @ v) @ W_o + out_bias

Sharding: rows (first pair axis) split across 8 cores, 48 rows each; each
core computes its bias shard, AllGather produces the full [h,N,N] bias
(FastFold DAP-style gather).

Per-core implementation:
  - Phase 1 streams rows: LayerNorm stats in [tok,d] layout, normalize,
    PE-transpose into a resident bf16 Z^T [d, R*N], project bias [4,N]
    per row, DMA to the AllGather shard.  rsqrt = Sqrt + DVE reciprocal
    (keeps ACT on one table set; Ln/Exp split across sets thrashes).
  - exp(bias^T) precomputed once so the softmax bias-add becomes a bf16
    multiply after exp: exp(l+b) = exp(l)*exp(b).
  - Phase 2 per row: q/k/gate projections in [hc,tok], v in [tok,hc];
    QK^T computed transposed ([k,q]) with the 4 heads (K=c=32) packed
    via tile_position row groups; softmax sums via 2.0-valued ones
    matmuls col-packed per head (the factor 2 absorbs the 0.5 of
    sigmoid(x) = (1+tanh(x/2))/2 — tanh shares exp's ACT table set);
    normalization by reciprocal_approx_fast; output projection uses gwa
    chunks as the stationary operand producing [tok,d] directly (no
    fp32 transposes), then residual + out_bias adds.
"""

import os
import sys

for _p in ("/opt/trn_rl_repo",):
    if _p not in sys.path:
        sys.path.append(_p)

import numpy as np
import ml_dtypes

import concourse.bass as bass
import concourse.bacc as bacc
import concourse.tile as tile
from concourse import mybir

F32 = mybir.dt.float32
BF16 = mybir.dt.bfloat16
AF = mybir.ActivationFunctionType
ALU = mybir.AluOpType
AX = mybir.AxisListType

# incremental-feature flags (all algorithmic swaps default ON; structural
# PSUM re-layouts default OFF until proven hang-free on HW)
F_SQRT_LN = os.environ.get("K_SQRT_LN", "1") == "1"
F_TANH = os.environ.get("K_TANH", "1") == "1"
F_APPROX = os.environ.get("K_APPROX", "1") == "1"
F_DIRECT_OUT = os.environ.get("K_DIRECT_OUT", "1") == "1"
# tensor_tensor_reduce hangs TRN2 hardware here (sim passes) — keep off
F_TTR = os.environ.get("K_TTR", "0") == "1"
# shared multi-bank PSUM tiles + batched exp/mul (fewer, larger ops)
F_BIG = os.environ.get("K_BIG", "1") == "1"

P = 128          # partitions
D = 128          # d_pair
NH = 4           # heads
CH = 32          # head dim
HC = NH * CH     # 128


def build_nc(N=384, n_cores=8):
    C3 = N // P           # chunks along the attention axis
    R = N // n_cores      # rows per core

    nc = bacc.Bacc(
        "TRN2",
        target_bir_lowering=False,
        debug=False,
        enable_asserts=False,
        num_devices=n_cores,
    )

    Zr = nc.dram_tensor("z_raw", [R, N, D], F32, kind="ExternalInput").ap()
    Zm = nc.dram_tensor("z_mask", [R, N], F32, kind="ExternalInput").ap()
    lnw_d = nc.dram_tensor("ln_w", [D], F32, kind="ExternalInput").ap()
    lnb_d = nc.dram_tensor("ln_b", [D], F32, kind="ExternalInput").ap()
    wb_d = nc.dram_tensor("w_b", [D, NH], F32, kind="ExternalInput").ap()
    wqkv_d = nc.dram_tensor("w_qkv", [D, 3 * HC], F32, kind="ExternalInput").ap()
    wg_d = nc.dram_tensor("w_gate", [D, HC], F32, kind="ExternalInput").ap()
    gb_d = nc.dram_tensor("gating_bias", [HC], F32, kind="ExternalInput").ap()
    wo_d = nc.dram_tensor("w_o", [HC, D], F32, kind="ExternalInput").ap()
    ob_d = nc.dram_tensor("out_bias", [D], F32, kind="ExternalInput").ap()
    OUT = nc.dram_tensor("out", [R, N, D], F32, kind="ExternalOutput").ap()

    id_bf_d = nc.inline_tensor(np.eye(P, dtype=ml_dtypes.bfloat16), "id_bf_c").ap()
    id_f_d = nc.inline_tensor(np.eye(P, dtype=np.float32), "id_f_c").ap()
    sums_w = 2.0 if F_TANH else 1.0
    ones_d = nc.inline_tensor(
        np.full((P, CH), sums_w, dtype=ml_dtypes.bfloat16), "ones_c"
    ).ap()
    obb_np = np.zeros((P, P), dtype=np.float32)  # placeholder, filled on-device

    with tile.TileContext(nc) as tc:
        with (
            tc.tile_pool(name="const", bufs=1) as constp,
            tc.tile_pool(name="res", bufs=1) as resp,
            tc.tile_pool(name="work", bufs=2) as work,
            tc.tile_pool(name="stat", bufs=3) as statp,
            tc.tile_pool(name="wpool", bufs=4) as wpool,
            tc.tile_pool(name="ps", bufs=1, space="PSUM") as psum,
            tc.tile_pool(name="dram", bufs=1, space="DRAM") as dramp,
        ):
            # ---- constants / weights ----
            id_bf = constp.tile([P, P], BF16)
            nc.sync.dma_start(id_bf, id_bf_d)
            id_f = constp.tile([P, P], F32)
            nc.sync.dma_start(id_f, id_f_d)
            ones_bf = constp.tile([P, CH], BF16)
            nc.sync.dma_start(ones_bf, ones_d)

            lnw = constp.tile([D, 1], F32)
            nc.sync.dma_start(lnw, lnw_d[:, None])
            lnb = constp.tile([D, 1], F32)
            nc.sync.dma_start(lnb, lnb_d[:, None])
            ob = constp.tile([D, 1], F32)
            nc.sync.dma_start(ob, ob_d[:, None])
            gb = constp.tile([HC, 1], F32)
            nc.sync.dma_start(gb, gb_d[:, None])
            ngb = constp.tile([HC, 1], F32)
            nc.scalar.mul(ngb, gb, 0.5 if F_TANH else -1.0)
            eps_c = constp.tile([P, 1], F32)
            nc.gpsimd.memset(eps_c, 1e-5)
            neg1e9_c = constp.tile([P, 1], F32)
            nc.gpsimd.memset(neg1e9_c, -1e9)

            wtmp = constp.tile([D, 3 * HC], F32, tag="wtmp")
            nc.sync.dma_start(wtmp, wqkv_d)
            wq = constp.tile([D, HC], BF16)
            nc.scalar.activation(wq, wtmp[:, 0:HC], AF.Copy, scale=CH ** -0.5)
            wk = constp.tile([D, HC], BF16)
            nc.scalar.copy(wk, wtmp[:, HC:2 * HC])
            wv = constp.tile([D, HC], BF16)
            nc.scalar.copy(wv, wtmp[:, 2 * HC:3 * HC])

            wgt = constp.tile([D, HC], F32, tag="wgt")
            nc.sync.dma_start(wgt, wg_d)
            wg = constp.tile([D, HC], BF16)
            nc.scalar.copy(wg, wgt)
            wot = constp.tile([HC, D], F32, tag="wot")
            nc.sync.dma_start(wot, wo_d)
            wo = constp.tile([HC, D], BF16)
            nc.scalar.copy(wo, wot)
            wbt = constp.tile([D, NH], F32, tag="wbt")
            nc.sync.dma_start(wbt, wb_d)
            wb = constp.tile([D, NH], BF16)
            nc.scalar.copy(wb, wbt)

            # out_bias broadcast [tok, d] for the residual stage
            obb = constp.tile([P, P], F32)
            if F_DIRECT_OUT:
                obr = constp.tile([1, D], F32)
                nc.sync.dma_start(obr, ob_d[None, :])
                ones1 = constp.tile([1, P], F32)
                nc.gpsimd.memset(ones1, 1.0)
                obp = psum.tile(
                    [P, P], F32,
                    tag="out" if F_BIG else "logits",
                    bufs=2 if F_BIG else 3,
                    name="obp",
                )
                nc.tensor.matmul(obp, ones1, obr)
                nc.scalar.copy(obb, obp)

            # mask bias columns: mb[kc][k, i] = (Z_mask[i, k] - 1) * 1e9
            mb = []
            for kc in range(C3):
                mk = work.tile([P, R], F32, tag="mk")
                nc.sync.dma_start(
                    mk, Zm[:, kc * P:(kc + 1) * P].rearrange("r p -> p r")
                )
                mbt = resp.tile([P, R], F32, tag=f"mb{kc}", name=f"mb{kc}")
                nc.scalar.activation(mbt, mk, AF.Identity, scale=1e9, bias=neg1e9_c)
                mb.append(mbt)

            # DRAM bounce buffers for the bias AllGather
            b_shard = dramp.tile([R, NH, N], BF16, tag="bshard")
            b_full = dramp.tile(
                [n_cores * R, NH, N], BF16, tag="bfull", addr_space="Shared"
            )

            # ---- phase 1: LayerNorm -> resident Z^T, bias shard ----
            Zt = resp.tile([P, R * C3 * P], BF16, tag="Zt")
            for q in range(R):
                zrow = work.tile([P, C3, P], F32, tag="zrow")
                nc.sync.dma_start(zrow, Zr[q].rearrange("(c p) d -> p c d", p=P))
                s1 = statp.tile([P, C3], F32, tag="s1")
                nc.vector.reduce_sum(s1, zrow, axis=AX.X)
                s2 = statp.tile([P, C3], F32, tag="s2")
                if F_TTR:
                    scr = work.tile([P, P], F32, tag="scr")
                    for c in range(C3):
                        nc.vector.tensor_tensor_reduce(
                            scr, zrow[:, c, :], zrow[:, c, :],
                            scale=1.0, scalar=0.0,
                            op0=ALU.mult, op1=ALU.add,
                            accum_out=s2[:, c:c + 1],
                        )
                else:
                    zsq = work.tile([P, C3, P], F32, tag="zsq")
                    nc.scalar.square(zsq, zrow)
                    nc.vector.reduce_sum(s2, zsq, axis=AX.X)
                mu = statp.tile([P, C3], F32, tag="mu")
                nc.scalar.mul(mu, s1, 1.0 / D)
                musq = statp.tile([P, C3], F32, tag="musq")
                nc.vector.tensor_mul(musq, mu, mu)
                var = statp.tile([P, C3], F32, tag="var")
                nc.vector.scalar_tensor_tensor(
                    var, s2, 1.0 / D, musq, op0=ALU.mult, op1=ALU.subtract
                )
                rsig = statp.tile([P, C3], F32, tag="rsig")
                if F_SQRT_LN:
                    std = statp.tile([P, C3], F32, tag="std")
                    nc.scalar.activation(std, var, AF.Sqrt, bias=eps_c)
                    nc.vector.reciprocal(rsig, std)
                else:
                    lv = statp.tile([P, C3], F32, tag="lv")
                    nc.scalar.activation(lv, var, AF.Ln, bias=eps_c)
                    nc.scalar.activation(rsig, lv, AF.Exp, scale=-0.5)
                nmr = statp.tile([P, C3], F32, tag="nmr")
                nc.vector.scalar_tensor_tensor(
                    nmr, mu, -1.0, rsig, op0=ALU.mult, op1=ALU.mult
                )
                if F_BIG:
                    tp = psum.tile([P, C3, P], BF16, tag="out", bufs=2, name="tp")
                    for c in range(C3):
                        zn = work.tile([P, P], BF16, tag="zn")
                        nc.vector.tensor_scalar(
                            zn, zrow[:, c, :], rsig[:, c:c + 1], nmr[:, c:c + 1],
                            op0=ALU.mult, op1=ALU.add,
                        )
                        nc.tensor.transpose(tp[:, c, :], zn, id_bf)
                    nc.vector.tensor_scalar(
                        Zt[:, q * C3 * P:(q + 1) * C3 * P].rearrange(
                            "p (c q2) -> p c q2", c=C3
                        ),
                        tp, lnw, lnb, op0=ALU.mult, op1=ALU.add,
                    )
                else:
                    for c in range(C3):
                        zn = work.tile([P, P], BF16, tag="zn")
                        nc.vector.tensor_scalar(
                            zn, zrow[:, c, :], rsig[:, c:c + 1], nmr[:, c:c + 1],
                            op0=ALU.mult, op1=ALU.add,
                        )
                        tp = psum.tile([P, P], BF16, tag="logits", bufs=3, name="tp")
                        nc.tensor.transpose(tp, zn, id_bf)
                        nc.vector.tensor_scalar(
                            Zt[:, (q * C3 + c) * P:(q * C3 + c + 1) * P],
                            tp, lnw, lnb, op0=ALU.mult, op1=ALU.add,
                        )
                bp = psum.tile(
                    [NH, N], F32,
                    tag="acc" if F_BIG else "sum",
                    bufs=1, name="bp",
                )
                nc.tensor.matmul(bp, wb, Zt[:, q * C3 * P:(q + 1) * C3 * P])
                bsb = work.tile([NH, N], BF16, tag="bsb")
                nc.vector.tensor_copy(bsb, bp)
                nc.sync.dma_start(b_shard[q], bsb)

            nc.gpsimd.collective_compute(
                "AllGather",
                ALU.bypass,
                replica_groups=[list(range(n_cores))],
                ins=[b_shard.opt()],
                outs=[b_full.opt()],
            )

            # exp of transposed bias, resident per k-chunk: Eb[kc][k, h, q]
            Eb = [
                resp.tile([P, NH, N], BF16, tag=f"eb{kc}", name=f"eb{kc}")
                for kc in range(C3)
            ]
            for qc in range(C3):
                bt = work.tile([P, NH, N], BF16, tag="bt")
                nc.sync.dma_start(bt, b_full[qc * P:(qc + 1) * P])
                for h in range(NH):
                    for kc in range(C3):
                        tp2 = psum.tile(
                            [P, P], BF16,
                            tag="out" if F_BIG else "logits",
                            bufs=2 if F_BIG else 3,
                            name="tp2",
                        )
                        nc.tensor.transpose(
                            tp2, bt[:, h, kc * P:(kc + 1) * P], id_bf
                        )
                        nc.scalar.activation(
                            Eb[kc][:, h, qc * P:(qc + 1) * P], tp2, AF.Exp
                        )

            # ---- phase 2: per-row attention ----
            for i in range(R):
                zrow2 = work.tile([P, C3, P], F32, tag="zrow2")
                nc.sync.dma_start(zrow2, Zr[i].rearrange("(c p) d -> p c d", p=P))
                zt_row = Zt[:, i * C3 * P:(i + 1) * C3 * P]

                if F_BIG:
                    # q/k/gate [hc,tok] + v [tok,hc] share one 4-bank psum tile
                    pj = psum.tile([P, NH, 512], F32, tag="big", bufs=1, name="pj")
                    nc.tensor.matmul(pj[:, 0, 0:N], wq, zt_row)
                    nc.tensor.matmul(pj[:, 1, 0:N], wk, zt_row)
                    nc.tensor.matmul(pj[:, 2, 0:N], wg, zt_row)
                    for c in range(C3):
                        nc.tensor.matmul(
                            pj[:, 3, c * P:(c + 1) * P],
                            zt_row[:, c * P:(c + 1) * P],
                            wv,
                        )
                    qk_sb = work.tile([P, 2, N], BF16, tag="qk_sb")
                    nc.vector.tensor_copy(qk_sb, pj[:, 0:2, 0:N])
                    qt = qk_sb[:, 0, :]
                    kt = qk_sb[:, 1, :]
                    gp = pj[:, 2, 0:N]
                    vsb3 = work.tile([P, C3, P], BF16, tag="vsb")
                    nc.vector.tensor_copy(
                        vsb3.rearrange("p c q2 -> p (c q2)"), pj[:, 3, 0:N]
                    )
                else:
                    qp = psum.tile([P, N], F32, tag="proj", bufs=2, name="qp")
                    nc.tensor.matmul(qp, wq, zt_row)
                    qt = work.tile([P, N], BF16, tag="qt")
                    nc.vector.tensor_copy(qt, qp)
                    kp = psum.tile([P, N], F32, tag="proj", bufs=2, name="kp")
                    nc.tensor.matmul(kp, wk, zt_row)
                    kt = work.tile([P, N], BF16, tag="kt")
                    nc.vector.tensor_copy(kt, kp)
                    gpt = psum.tile([P, N], F32, tag="proj", bufs=2, name="gpt")
                    nc.tensor.matmul(gpt, wg, zt_row)
                    gp = gpt
                    vp = psum.tile([P, C3, P], F32, tag="proj", bufs=2, name="vp")
                    for c in range(C3):
                        nc.tensor.matmul(
                            vp[:, c, :],
                            zt_row[:, c * P:(c + 1) * P],
                            wv,
                        )
                    vsb3 = work.tile([P, C3, P], BF16, tag="vsb")
                    nc.vector.tensor_copy(vsb3, vp)

                if F_TANH:
                    th = work.tile([P, N], BF16, tag="th")
                    nc.scalar.activation(th, gp, AF.Tanh, scale=0.5, bias=ngb)
                else:
                    eg = work.tile([P, N], BF16, tag="eg")
                    nc.scalar.activation(eg, gp, AF.Exp, scale=-1.0, bias=ngb)
                    g1 = work.tile([P, N], F32, tag="g1")
                    nc.vector.tensor_scalar_add(g1, eg, 1.0)
                    gate = work.tile([P, N], F32, tag="gate")
                    nc.vector.reciprocal(gate, g1)

                if F_BIG:
                    wap3 = psum.tile([P, 2, 512], F32, tag="acc", bufs=1, name="wap3")
                    wap = wap3[:, 0, 0:N]
                    sp = wap3[:, 1, 0:N]
                else:
                    wapt = psum.tile([P, N], F32, tag="wa", bufs=1, name="wapt")
                    spt = psum.tile([P, N], F32, tag="sum", bufs=1, name="spt")
                    wap, sp = wapt, spt

                for kc in range(C3):
                    if F_BIG:
                        lg4 = psum.tile([P, NH, 512], F32, tag="big", bufs=1, name="lg4")
                        for h in range(NH):
                            nc.tensor.matmul(
                                lg4[:, h, 0:N],
                                kt[CH * h:CH * (h + 1), kc * P:(kc + 1) * P],
                                qt[CH * h:CH * (h + 1), :],
                                tile_position=(CH * h, 0),
                            )
                        w4 = wpool.tile([P, NH, N], BF16, tag="wt")
                        nc.scalar.activation(
                            w4, lg4[:, :, 0:N], AF.Exp, bias=mb[kc][:, i:i + 1]
                        )
                        wm4 = wpool.tile([P, NH, N], BF16, tag="wm")
                        nc.vector.tensor_mul(wm4, w4, Eb[kc])
                        wms = [wm4[:, h, :] for h in range(NH)]
                    else:
                        wms = []
                        for h in range(NH):
                            lg = psum.tile([P, N], F32, tag="logits", bufs=3, name="lg")
                            nc.tensor.matmul(
                                lg,
                                kt[CH * h:CH * (h + 1), kc * P:(kc + 1) * P],
                                qt[CH * h:CH * (h + 1), :],
                                tile_position=(CH * h, 0),
                            )
                            w_t = wpool.tile([P, N], BF16, tag="wt")
                            nc.scalar.activation(
                                w_t, lg, AF.Exp, bias=mb[kc][:, i:i + 1]
                            )
                            wm = wpool.tile([P, N], BF16, tag="wm")
                            nc.vector.tensor_mul(wm, w_t, Eb[kc][:, h, :])
                            wms.append(wm)
                    for h in range(NH):
                        nc.tensor.matmul(
                            wap[CH * h:CH * (h + 1), :],
                            vsb3[:, kc, CH * h:CH * (h + 1)],
                            wms[h],
                            start=(kc == 0),
                            stop=(kc == C3 - 1),
                            skip_group_check=True,
                            tile_position=(0, CH * h),
                        )
                    for h in range(NH):
                        nc.tensor.matmul(
                            sp[CH * h:CH * (h + 1), :],
                            ones_bf,
                            wms[h],
                            start=(kc == 0),
                            stop=(kc == C3 - 1),
                            skip_group_check=True,
                            tile_position=(0, CH * h),
                        )

                rs = work.tile([P, N], F32, tag="rs")
                if F_APPROX:
                    nc.vector.reciprocal_approx_fast(rs, sp)
                else:
                    nc.vector.reciprocal(rs, sp)
                wan = work.tile([P, N], F32, tag="wan")
                nc.vector.tensor_mul(wan, wap, rs)
                gwa = work.tile([P, N], BF16, tag="gwa")
                if F_TANH:
                    # gwa = (tanh+1) * (wa / 2s) == sigmoid * wa / s
                    nc.vector.scalar_tensor_tensor(
                        gwa, th, 1.0, wan, op0=ALU.add, op1=ALU.mult
                    )
                else:
                    nc.vector.tensor_mul(gwa, wan, gate)

                if F_DIRECT_OUT:
                    out_ps = psum.tile(
                        [P, C3, P], F32,
                        tag="out" if F_BIG else "oproj",
                        bufs=2 if F_BIG else 1,
                        name="out_ps",
                    )
                    for c in range(C3):
                        nc.tensor.matmul(
                            out_ps[:, c, :], gwa[:, c * P:(c + 1) * P], wo
                        )
                    fin = work.tile([P, C3, P], F32, tag="fin")
                    nc.vector.tensor_add(fin, out_ps, zrow2)
                    for c in range(C3):
                        nc.vector.tensor_add(fin[:, c, :], fin[:, c, :], obb)
                else:
                    op_ = psum.tile([P, N], F32, tag="oproj", bufs=1, name="op_")
                    nc.tensor.matmul(op_, wo, gwa)
                    osb = work.tile([P, N], F32, tag="osb")
                    nc.scalar.activation(osb, op_, AF.Identity, bias=ob)
                    ot = psum.tile([P, C3, P], F32, tag="proj", bufs=2, name="ot")
                    for c in range(C3):
                        nc.tensor.transpose(
                            ot[:, c, :], osb[:, c * P:(c + 1) * P], id_f
                        )
                    fin = work.tile([P, C3, P], F32, tag="fin")
                    nc.vector.tensor_add(fin, ot, zrow2)
                nc.sync.dma_start(OUT[i].rearrange("(c p) d -> p c d", p=P), fin)

    nc.compile()
    return nc


_CACHE = {}


def get_nc(N=384, n_cores=8):
    key = (N, n_cores)
    if key not in _CACHE:
        _CACHE[key] = build_nc(N, n_cores)
    return _CACHE[key]


def make_in_maps(inputs, N=384, n_cores=8):
    R = N // n_cores
    Z = np.ascontiguousarray(np.asarray(inputs["Z_raw"], dtype=np.float32))
    M = np.ascontiguousarray(np.asarray(inputs["Z_mask"], dtype=np.float32))
    shared = {
        "ln_w": np.ascontiguousarray(np.asarray(inputs["ln_w"], np.float32)),
        "ln_b": np.ascontiguousarray(np.asarray(inputs["ln_b"], np.float32)),
        "w_b": np.ascontiguousarray(np.asarray(inputs["W_b"], np.float32)),
        "w_qkv": np.ascontiguousarray(np.asarray(inputs["W_qkv"], np.float32)),
        "w_gate": np.ascontiguousarray(np.asarray(inputs["W_gate"], np.float32)),
        "gating_bias": np.ascontiguousarray(
            np.asarray(inputs["gating_bias"], np.float32)
        ),
        "w_o": np.ascontiguousarray(np.asarray(inputs["W_o"], np.float32)),
        "out_bias": np.ascontiguousarray(np.asarray(inputs["out_bias"], np.float32)),
    }
    in_maps = []
    for c in range(n_cores):
        m = dict(shared)
        m["z_raw"] = np.ascontiguousarray(Z[0, c * R:(c + 1) * R])
        m["z_mask"] = np.ascontiguousarray(M[0, c * R:(c + 1) * R])
        in_maps.append(m)
    return in_maps


def kernel(**inputs):
    from concourse.bass_utils import run_bass_kernel_spmd

    N, n_cores = 384, 8
    nc = get_nc(N, n_cores)
    in_maps = make_in_maps(inputs, N, n_cores)
    res = run_bass_kernel_spmd(nc, in_maps, list(range(n_cores)))
    out = np.concatenate([res.results[c]["out"] for c in range(n_cores)], axis=0)
    return out.reshape(1, N, N, D).astype(np.float32)


# revision 25
# speedup vs baseline: 1.1070x; 1.1070x over previous
"""Trainium2 Bass kernel for ChunkTriangleAttentionStartingNode.

Computation (B=1, N=384, D=128, h=4, c=32):
  Z = LayerNorm(Z_raw) * ln_w + ln_b                     (over d_pair)
  bias[h,q,k]   = (Z @ W_b)[q,k,h]        (triangle bias, row-indexed by q)
  q,k,v         = split(Z @ W_qkv)        per pair-row i, heads h, dim c
  logits[i,h,q,k] = q.k / sqrt(c) + mask_bias[i,k] + bias[h,q,k]
  out = Z_raw + (sigmoid(Z@W_gate + gb) * softmax(logits) @ v) @ W_o + out_bias

Sharding: rows (first pair axis) split across 8 cores, 48 rows each; each
core computes its bias shard, AllGather produces the full [h,N,N] bias
(FastFold DAP-style gather).

Per-core implementation:
  - Phase 1 streams rows: LayerNorm stats in [tok,d] layout, normalize,
    PE-transpose into a resident bf16 Z^T [d, R*N], project bias [4,N]
    per row, DMA to the AllGather shard.  rsqrt = Sqrt + DVE reciprocal
    (keeps ACT on one table set; Ln/Exp split across sets thrashes).
  - exp(bias^T) precomputed once so the softmax bias-add becomes a bf16
    multiply after exp: exp(l+b) = exp(l)*exp(b).
  - Phase 2 per row: q/k/gate projections in [hc,tok], v in [tok,hc];
    QK^T computed transposed ([k,q]) with the 4 heads (K=c=32) packed
    via tile_position row groups; softmax sums via 2.0-valued ones
    matmuls col-packed per head (the factor 2 absorbs the 0.5 of
    sigmoid(x) = (1+tanh(x/2))/2 — tanh shares exp's ACT table set);
    normalization by reciprocal_approx_fast; output projection uses gwa
    chunks as the stationary operand producing [tok,d] directly (no
    fp32 transposes), then residual + out_bias adds.
"""

import os
import sys

for _p in ("/opt/trn_rl_repo",):
    if _p not in sys.path:
        sys.path.append(_p)

import numpy as np
import ml_dtypes

import concourse.bass as bass
import concourse.bacc as bacc
import concourse.tile as tile
from concourse import mybir

F32 = mybir.dt.float32
BF16 = mybir.dt.bfloat16
AF = mybir.ActivationFunctionType
ALU = mybir.AluOpType
AX = mybir.AxisListType

# incremental-feature flags (all algorithmic swaps default ON; structural
# PSUM re-layouts default OFF until proven hang-free on HW)
F_SQRT_LN = os.environ.get("K_SQRT_LN", "1") == "1"
F_TANH = os.environ.get("K_TANH", "1") == "1"
F_APPROX = os.environ.get("K_APPROX", "1") == "1"
F_DIRECT_OUT = os.environ.get("K_DIRECT_OUT", "1") == "1"
# tensor_tensor_reduce hangs TRN2 hardware here (sim passes) — keep off
F_TTR = os.environ.get("K_TTR", "0") == "1"
# shared multi-bank PSUM tiles + batched exp/mul (fewer, larger ops)
F_BIG = os.environ.get("K_BIG", "1") == "1"

P = 128          # partitions
D = 128          # d_pair
NH = 4           # heads
CH = 32          # head dim
HC = NH * CH     # 128


def build_nc(N=384, n_cores=8):
    C3 = N // P           # chunks along the attention axis
    R = N // n_cores      # rows per core

    nc = bacc.Bacc(
        "TRN2",
        target_bir_lowering=False,
        debug=False,
        enable_asserts=False,
        num_devices=n_cores,
    )

    Zr = nc.dram_tensor("z_raw", [R, N, D], F32, kind="ExternalInput").ap()
    Zm = nc.dram_tensor("z_mask", [R, N], F32, kind="ExternalInput").ap()
    lnw_d = nc.dram_tensor("ln_w", [D], F32, kind="ExternalInput").ap()
    lnb_d = nc.dram_tensor("ln_b", [D], F32, kind="ExternalInput").ap()
    wb_d = nc.dram_tensor("w_b", [D, NH], F32, kind="ExternalInput").ap()
    wqkv_d = nc.dram_tensor("w_qkv", [D, 3 * HC], F32, kind="ExternalInput").ap()
    wg_d = nc.dram_tensor("w_gate", [D, HC], F32, kind="ExternalInput").ap()
    gb_d = nc.dram_tensor("gating_bias", [HC], F32, kind="ExternalInput").ap()
    wo_d = nc.dram_tensor("w_o", [HC, D], F32, kind="ExternalInput").ap()
    ob_d = nc.dram_tensor("out_bias", [D], F32, kind="ExternalInput").ap()
    OUT = nc.dram_tensor("out", [R, N, D], F32, kind="ExternalOutput").ap()

    id_bf_d = nc.inline_tensor(np.eye(P, dtype=ml_dtypes.bfloat16), "id_bf_c").ap()
    id_f_d = nc.inline_tensor(np.eye(P, dtype=np.float32), "id_f_c").ap()
    sums_w = 2.0 if F_TANH else 1.0
    ones_d = nc.inline_tensor(
        np.full((P, CH), sums_w, dtype=ml_dtypes.bfloat16), "ones_c"
    ).ap()
    obb_np = np.zeros((P, P), dtype=np.float32)  # placeholder, filled on-device

    with tile.TileContext(nc) as tc:
        with (
            tc.tile_pool(name="const", bufs=1) as constp,
            tc.tile_pool(name="res", bufs=1) as resp,
            tc.tile_pool(name="work", bufs=2) as work,
            tc.tile_pool(name="stat", bufs=3) as statp,
            tc.tile_pool(name="wpool", bufs=4) as wpool,
            tc.tile_pool(name="ps", bufs=1, space="PSUM") as psum,
            tc.tile_pool(name="dram", bufs=1, space="DRAM") as dramp,
        ):
            # ---- constants / weights ----
            id_bf = constp.tile([P, P], BF16)
            nc.sync.dma_start(id_bf, id_bf_d)
            id_f = constp.tile([P, P], F32)
            nc.sync.dma_start(id_f, id_f_d)
            ones_bf = constp.tile([P, CH], BF16)
            nc.sync.dma_start(ones_bf, ones_d)

            lnw = constp.tile([D, 1], F32)
            nc.sync.dma_start(lnw, lnw_d[:, None])
            lnb = constp.tile([D, 1], F32)
            nc.sync.dma_start(lnb, lnb_d[:, None])
            ob = constp.tile([D, 1], F32)
            nc.sync.dma_start(ob, ob_d[:, None])
            gb = constp.tile([HC, 1], F32)
            nc.sync.dma_start(gb, gb_d[:, None])
            ngb = constp.tile([HC, 1], F32)
            nc.scalar.mul(ngb, gb, 0.5 if F_TANH else -1.0)
            eps_c = constp.tile([P, 1], F32)
            nc.gpsimd.memset(eps_c, 1e-5)
            neg1e9_c = constp.tile([P, 1], F32)
            nc.gpsimd.memset(neg1e9_c, -1e9)

            wtmp = constp.tile([D, 3 * HC], F32, tag="wtmp")
            nc.sync.dma_start(wtmp, wqkv_d)
            wq = constp.tile([D, HC], BF16)
            nc.scalar.activation(wq, wtmp[:, 0:HC], AF.Copy, scale=CH ** -0.5)
            wk = constp.tile([D, HC], BF16)
            nc.scalar.copy(wk, wtmp[:, HC:2 * HC])
            wv = constp.tile([D, HC], BF16)
            nc.scalar.copy(wv, wtmp[:, 2 * HC:3 * HC])

            wgt = constp.tile([D, HC], F32, tag="wgt")
            nc.sync.dma_start(wgt, wg_d)
            wg = constp.tile([D, HC], BF16)
            nc.scalar.copy(wg, wgt)
            wot = constp.tile([HC, D], F32, tag="wot")
            nc.sync.dma_start(wot, wo_d)
            wo = constp.tile([HC, D], BF16)
            nc.scalar.copy(wo, wot)
            wbt = constp.tile([D, NH], F32, tag="wbt")
            nc.sync.dma_start(wbt, wb_d)
            wb = constp.tile([D, NH], BF16)
            nc.scalar.copy(wb, wbt)

            # out_bias broadcast [tok, d] for the residual stage
            obb = constp.tile([P, P], F32)
            if F_DIRECT_OUT:
                obr = constp.tile([1, D], F32)
                nc.sync.dma_start(obr, ob_d[None, :])
                ones1 = constp.tile([1, P], F32)
                nc.gpsimd.memset(ones1, 1.0)
                obp = psum.tile(
                    [P, P], F32,
                    tag="out" if F_BIG else "logits",
                    bufs=2 if F_BIG else 3,
                    name="obp",
                )
                nc.tensor.matmul(obp, ones1, obr)
                nc.scalar.copy(obb, obp)

            # mask bias columns: mb[kc][k, i] = (Z_mask[i, k] - 1) * 1e9
            mb = []
            for kc in range(C3):
                mk = work.tile([P, R], F32, tag="mk")
                nc.sync.dma_start(
                    mk, Zm[:, kc * P:(kc + 1) * P].rearrange("r p -> p r")
                )
                mbt = resp.tile([P, R], F32, tag=f"mb{kc}", name=f"mb{kc}")
                nc.scalar.activation(mbt, mk, AF.Identity, scale=1e9, bias=neg1e9_c)
                mb.append(mbt)

            # DRAM bounce buffers for the bias AllGather
            b_shard = dramp.tile([R, NH, N], BF16, tag="bshard")
            b_full = dramp.tile(
                [n_cores * R, NH, N], BF16, tag="bfull", addr_space="Shared"
            )

            # ---- phase 1: LayerNorm -> resident Z^T, bias shard ----
            Zt = resp.tile([P, R * C3 * P], BF16, tag="Zt")
            for q in range(R):
                zrow = work.tile([P, C3, P], F32, tag="zrow")
                nc.sync.dma_start(zrow, Zr[q].rearrange("(c p) d -> p c d", p=P))
                s1 = statp.tile([P, C3], F32, tag="s1")
                nc.vector.reduce_sum(s1, zrow, axis=AX.X)
                s2 = statp.tile([P, C3], F32, tag="s2")
                if F_TTR:
                    scr = work.tile([P, P], F32, tag="scr")
                    for c in range(C3):
                        nc.vector.tensor_tensor_reduce(
                            scr, zrow[:, c, :], zrow[:, c, :],
                            scale=1.0, scalar=0.0,
                            op0=ALU.mult, op1=ALU.add,
                            accum_out=s2[:, c:c + 1],
                        )
                else:
                    zsq = work.tile([P, C3, P], F32, tag="zsq")
                    nc.scalar.square(zsq, zrow)
                    nc.vector.reduce_sum(s2, zsq, axis=AX.X)
                mu = statp.tile([P, C3], F32, tag="mu")
                nc.scalar.mul(mu, s1, 1.0 / D)
                musq = statp.tile([P, C3], F32, tag="musq")
                nc.vector.tensor_mul(musq, mu, mu)
                var = statp.tile([P, C3], F32, tag="var")
                nc.vector.scalar_tensor_tensor(
                    var, s2, 1.0 / D, musq, op0=ALU.mult, op1=ALU.subtract
                )
                rsig = statp.tile([P, C3], F32, tag="rsig")
                if F_SQRT_LN:
                    std = statp.tile([P, C3], F32, tag="std")
                    nc.scalar.activation(std, var, AF.Sqrt, bias=eps_c)
                    nc.vector.reciprocal(rsig, std)
                else:
                    lv = statp.tile([P, C3], F32, tag="lv")
                    nc.scalar.activation(lv, var, AF.Ln, bias=eps_c)
                    nc.scalar.activation(rsig, lv, AF.Exp, scale=-0.5)
                nmr = statp.tile([P, C3], F32, tag="nmr")
                nc.vector.scalar_tensor_tensor(
                    nmr, mu, -1.0, rsig, op0=ALU.mult, op1=ALU.mult
                )
                if F_BIG:
                    tp = psum.tile([P, C3, P], BF16, tag="out", bufs=2, name="tp")
                    for c in range(C3):
                        zn = work.tile([P, P], BF16, tag="zn")
                        nc.vector.tensor_scalar(
                            zn, zrow[:, c, :], rsig[:, c:c + 1], nmr[:, c:c + 1],
                            op0=ALU.mult, op1=ALU.add,
                        )
                        nc.tensor.transpose(tp[:, c, :], zn, id_bf)
                    nc.vector.tensor_scalar(
                        Zt[:, q * C3 * P:(q + 1) * C3 * P].rearrange(
                            "p (c q2) -> p c q2", c=C3
                        ),
                        tp, lnw, lnb, op0=ALU.mult, op1=ALU.add,
                    )
                else:
                    for c in range(C3):
                        zn = work.tile([P, P], BF16, tag="zn")
                        nc.vector.tensor_scalar(
                            zn, zrow[:, c, :], rsig[:, c:c + 1], nmr[:, c:c + 1],
                            op0=ALU.mult, op1=ALU.add,
                        )
                        tp = psum.tile([P, P], BF16, tag="logits", bufs=3, name="tp")
                        nc.tensor.transpose(tp, zn, id_bf)
                        nc.vector.tensor_scalar(
                            Zt[:, (q * C3 + c) * P:(q * C3 + c + 1) * P],
                            tp, lnw, lnb, op0=ALU.mult, op1=ALU.add,
                        )
                bp = psum.tile(
                    [NH, N], F32,
                    tag="acc" if F_BIG else "sum",
                    bufs=1, name="bp",
                )
                nc.tensor.matmul(bp, wb, Zt[:, q * C3 * P:(q + 1) * C3 * P])
                bsb = work.tile([NH, N], BF16, tag="bsb")
                nc.vector.tensor_copy(bsb, bp)
                nc.sync.dma_start(b_shard[q], bsb)

            nc.gpsimd.collective_compute(
                "AllGather",
                ALU.bypass,
                replica_groups=[list(range(n_cores))],
                ins=[b_shard.opt()],
                outs=[b_full.opt()],
            )

            # exp of transposed bias, resident per k-chunk: Eb[kc][k, h, q]
            Eb = [
                resp.tile([P, NH, N], BF16, tag=f"eb{kc}", name=f"eb{kc}")
                for kc in range(C3)
            ]
            for qc in range(C3):
                bt = work.tile([P, NH, N], BF16, tag="bt")
                nc.sync.dma_start(bt, b_full[qc * P:(qc + 1) * P])
                for h in range(NH):
                    for kc in range(C3):
                        tp2 = psum.tile(
                            [P, P], BF16,
                            tag="out" if F_BIG else "logits",
                            bufs=2 if F_BIG else 3,
                            name="tp2",
                        )
                        nc.tensor.transpose(
                            tp2, bt[:, h, kc * P:(kc + 1) * P], id_bf
                        )
                        nc.scalar.activation(
                            Eb[kc][:, h, qc * P:(qc + 1) * P], tp2, AF.Exp
                        )

            # ---- phase 2: per-row attention ----
            for i in range(R):
                zrow2 = work.tile([P, C3, P], F32, tag="zrow2")
                nc.sync.dma_start(zrow2, Zr[i].rearrange("(c p) d -> p c d", p=P))
                zt_row = Zt[:, i * C3 * P:(i + 1) * C3 * P]

                if F_BIG:
                    # two 2-bank tiles ping-pong: q/k in A, gate/v in B
                    pjA = psum.tile([P, 2, 512], F32, tag="bigA", bufs=1, name="pjA")
                    nc.tensor.matmul(pjA[:, 0, 0:N], wq, zt_row)
                    nc.tensor.matmul(pjA[:, 1, 0:N], wk, zt_row)
                    pjB = psum.tile([P, 2, 512], F32, tag="bigB", bufs=1, name="pjB")
                    nc.tensor.matmul(pjB[:, 0, 0:N], wg, zt_row)
                    for c in range(C3):
                        nc.tensor.matmul(
                            pjB[:, 1, c * P:(c + 1) * P],
                            zt_row[:, c * P:(c + 1) * P],
                            wv,
                        )
                    qk_sb = work.tile([P, 2, N], BF16, tag="qk_sb")
                    nc.vector.tensor_copy(qk_sb, pjA[:, 0:2, 0:N])
                    qt = qk_sb[:, 0, :]
                    kt = qk_sb[:, 1, :]
                    gp = pjB[:, 0, 0:N]
                    vsb3 = work.tile([P, C3, P], BF16, tag="vsb")
                    nc.vector.tensor_copy(
                        vsb3.rearrange("p c q2 -> p (c q2)"), pjB[:, 1, 0:N]
                    )
                else:
                    qp = psum.tile([P, N], F32, tag="proj", bufs=2, name="qp")
                    nc.tensor.matmul(qp, wq, zt_row)
                    qt = work.tile([P, N], BF16, tag="qt")
                    nc.vector.tensor_copy(qt, qp)
                    kp = psum.tile([P, N], F32, tag="proj", bufs=2, name="kp")
                    nc.tensor.matmul(kp, wk, zt_row)
                    kt = work.tile([P, N], BF16, tag="kt")
                    nc.vector.tensor_copy(kt, kp)
                    gpt = psum.tile([P, N], F32, tag="proj", bufs=2, name="gpt")
                    nc.tensor.matmul(gpt, wg, zt_row)
                    gp = gpt
                    vp = psum.tile([P, C3, P], F32, tag="proj", bufs=2, name="vp")
                    for c in range(C3):
                        nc.tensor.matmul(
                            vp[:, c, :],
                            zt_row[:, c * P:(c + 1) * P],
                            wv,
                        )
                    vsb3 = work.tile([P, C3, P], BF16, tag="vsb")
                    nc.vector.tensor_copy(vsb3, vp)

                if F_TANH:
                    th = work.tile([P, N], BF16, tag="th")
                    nc.scalar.activation(th, gp, AF.Tanh, scale=0.5, bias=ngb)
                else:
                    eg = work.tile([P, N], BF16, tag="eg")
                    nc.scalar.activation(eg, gp, AF.Exp, scale=-1.0, bias=ngb)
                    g1 = work.tile([P, N], F32, tag="g1")
                    nc.vector.tensor_scalar_add(g1, eg, 1.0)
                    gate = work.tile([P, N], F32, tag="gate")
                    nc.vector.reciprocal(gate, g1)

                if F_BIG:
                    wap3 = psum.tile([P, 2, 512], F32, tag="acc", bufs=1, name="wap3")
                    wap = wap3[:, 0, 0:N]
                    sp = wap3[:, 1, 0:N]
                else:
                    wapt = psum.tile([P, N], F32, tag="wa", bufs=1, name="wapt")
                    spt = psum.tile([P, N], F32, tag="sum", bufs=1, name="spt")
                    wap, sp = wapt, spt

                for kc in range(C3):
                    if F_BIG:
                        wms = []
                        for half, tag in ((0, "bigA"), (1, "bigB")):
                            lgH = psum.tile(
                                [P, 2, 512], F32, tag=tag, bufs=1, name=f"lg{tag}"
                            )
                            for hh in range(2):
                                h = half * 2 + hh
                                nc.tensor.matmul(
                                    lgH[:, hh, 0:N],
                                    kt[CH * h:CH * (h + 1), kc * P:(kc + 1) * P],
                                    qt[CH * h:CH * (h + 1), :],
                                    tile_position=(CH * h, 0),
                                )
                            wH = wpool.tile([P, 2, N], BF16, tag=f"wt{half}")
                            nc.scalar.activation(
                                wH, lgH[:, :, 0:N], AF.Exp, bias=mb[kc][:, i:i + 1]
                            )
                            wmH = wpool.tile([P, 2, N], BF16, tag=f"wm{half}")
                            nc.vector.tensor_mul(
                                wmH, wH, Eb[kc][:, 2 * half:2 * half + 2, :]
                            )
                            wms.extend([wmH[:, 0, :], wmH[:, 1, :]])
                    else:
                        wms = []
                        for h in range(NH):
                            lg = psum.tile([P, N], F32, tag="logits", bufs=3, name="lg")
                            nc.tensor.matmul(
                                lg,
                                kt[CH * h:CH * (h + 1), kc * P:(kc + 1) * P],
                                qt[CH * h:CH * (h + 1), :],
                                tile_position=(CH * h, 0),
                            )
                            w_t = wpool.tile([P, N], BF16, tag="wt")
                            nc.scalar.activation(
                                w_t, lg, AF.Exp, bias=mb[kc][:, i:i + 1]
                            )
                            wm = wpool.tile([P, N], BF16, tag="wm")
                            nc.vector.tensor_mul(wm, w_t, Eb[kc][:, h, :])
                            wms.append(wm)
                    for h in range(NH):
                        nc.tensor.matmul(
                            wap[CH * h:CH * (h + 1), :],
                            vsb3[:, kc, CH * h:CH * (h + 1)],
                            wms[h],
                            start=(kc == 0),
                            stop=(kc == C3 - 1),
                            skip_group_check=True,
                            tile_position=(0, CH * h),
                        )
                    for h in range(NH):
                        nc.tensor.matmul(
                            sp[CH * h:CH * (h + 1), :],
                            ones_bf,
                            wms[h],
                            start=(kc == 0),
                            stop=(kc == C3 - 1),
                            skip_group_check=True,
                            tile_position=(0, CH * h),
                        )

                rs = work.tile([P, N], F32, tag="rs")
                if F_APPROX:
                    nc.vector.reciprocal_approx_fast(rs, sp)
                else:
                    nc.vector.reciprocal(rs, sp)
                wan = work.tile([P, N], F32, tag="wan")
                nc.vector.tensor_mul(wan, wap, rs)
                gwa = work.tile([P, N], BF16, tag="gwa")
                if F_TANH:
                    # gwa = (tanh+1) * (wa / 2s) == sigmoid * wa / s
                    nc.vector.scalar_tensor_tensor(
                        gwa, th, 1.0, wan, op0=ALU.add, op1=ALU.mult
                    )
                else:
                    nc.vector.tensor_mul(gwa, wan, gate)

                if F_DIRECT_OUT:
                    out_ps = psum.tile(
                        [P, C3, P], F32,
                        tag="out" if F_BIG else "oproj",
                        bufs=2 if F_BIG else 1,
                        name="out_ps",
                    )
                    for c in range(C3):
                        nc.tensor.matmul(
                            out_ps[:, c, :], gwa[:, c * P:(c + 1) * P], wo
                        )
                    fin = work.tile([P, C3, P], F32, tag="fin")
                    nc.vector.tensor_add(fin, out_ps, zrow2)
                    for c in range(C3):
                        nc.vector.tensor_add(fin[:, c, :], fin[:, c, :], obb)
                else:
                    op_ = psum.tile([P, N], F32, tag="oproj", bufs=1, name="op_")
                    nc.tensor.matmul(op_, wo, gwa)
                    osb = work.tile([P, N], F32, tag="osb")
                    nc.scalar.activation(osb, op_, AF.Identity, bias=ob)
                    ot = psum.tile([P, C3, P], F32, tag="proj", bufs=2, name="ot")
                    for c in range(C3):
                        nc.tensor.transpose(
                            ot[:, c, :], osb[:, c * P:(c + 1) * P], id_f
                        )
                    fin = work.tile([P, C3, P], F32, tag="fin")
                    nc.vector.tensor_add(fin, ot, zrow2)
                nc.sync.dma_start(OUT[i].rearrange("(c p) d -> p c d", p=P), fin)

    nc.compile()
    return nc


_CACHE = {}


def get_nc(N=384, n_cores=8):
    key = (N, n_cores)
    if key not in _CACHE:
        _CACHE[key] = build_nc(N, n_cores)
    return _CACHE[key]


def make_in_maps(inputs, N=384, n_cores=8):
    R = N // n_cores
    Z = np.ascontiguousarray(np.asarray(inputs["Z_raw"], dtype=np.float32))
    M = np.ascontiguousarray(np.asarray(inputs["Z_mask"], dtype=np.float32))
    shared = {
        "ln_w": np.ascontiguousarray(np.asarray(inputs["ln_w"], np.float32)),
        "ln_b": np.ascontiguousarray(np.asarray(inputs["ln_b"], np.float32)),
        "w_b": np.ascontiguousarray(np.asarray(inputs["W_b"], np.float32)),
        "w_qkv": np.ascontiguousarray(np.asarray(inputs["W_qkv"], np.float32)),
        "w_gate": np.ascontiguousarray(np.asarray(inputs["W_gate"], np.float32)),
        "gating_bias": np.ascontiguousarray(
            np.asarray(inputs["gating_bias"], np.float32)
        ),
        "w_o": np.ascontiguousarray(np.asarray(inputs["W_o"], np.float32)),
        "out_bias": np.ascontiguousarray(np.asarray(inputs["out_bias"], np.float32)),
    }
    in_maps = []
    for c in range(n_cores):
        m = dict(shared)
        m["z_raw"] = np.ascontiguousarray(Z[0, c * R:(c + 1) * R])
        m["z_mask"] = np.ascontiguousarray(M[0, c * R:(c + 1) * R])
        in_maps.append(m)
    return in_maps


def kernel(**inputs):
    from concourse.bass_utils import run_bass_kernel_spmd

    N, n_cores = 384, 8
    nc = get_nc(N, n_cores)
    in_maps = make_in_maps(inputs, N, n_cores)
    res = run_bass_kernel_spmd(nc, in_maps, list(range(n_cores)))
    out = np.concatenate([res.results[c]["out"] for c in range(n_cores)], axis=0)
    return out.reshape(1, N, N, D).astype(np.float32)


# revision 26
# speedup vs baseline: 1.1396x; 1.0295x over previous
"""Trainium2 Bass kernel for ChunkTriangleAttentionStartingNode.

Computation (B=1, N=384, D=128, h=4, c=32):
  Z = LayerNorm(Z_raw) * ln_w + ln_b                     (over d_pair)
  bias[h,q,k]   = (Z @ W_b)[q,k,h]        (triangle bias, row-indexed by q)
  q,k,v         = split(Z @ W_qkv)        per pair-row i, heads h, dim c
  logits[i,h,q,k] = q.k / sqrt(c) + mask_bias[i,k] + bias[h,q,k]
  out = Z_raw + (sigmoid(Z@W_gate + gb) * softmax(logits) @ v) @ W_o + out_bias

Sharding: rows (first pair axis) split across 8 cores, 48 rows each; each
core computes its bias shard, AllGather produces the full [h,N,N] bias
(FastFold DAP-style gather).

Per-core implementation:
  - Phase 1 streams rows: LayerNorm stats in [tok,d] layout, normalize,
    PE-transpose into a resident bf16 Z^T [d, R*N], project bias [4,N]
    per row, DMA to the AllGather shard.  rsqrt = Sqrt + DVE reciprocal
    (keeps ACT on one table set; Ln/Exp split across sets thrashes).
  - exp(bias^T) precomputed once so the softmax bias-add becomes a bf16
    multiply after exp: exp(l+b) = exp(l)*exp(b).
  - Phase 2 per row: q/k/gate projections in [hc,tok], v in [tok,hc];
    QK^T computed transposed ([k,q]) with the 4 heads (K=c=32) packed
    via tile_position row groups; softmax sums via 2.0-valued ones
    matmuls col-packed per head (the factor 2 absorbs the 0.5 of
    sigmoid(x) = (1+tanh(x/2))/2 — tanh shares exp's ACT table set);
    normalization by reciprocal_approx_fast; output projection uses gwa
    chunks as the stationary operand producing [tok,d] directly (no
    fp32 transposes), then residual + out_bias adds.
"""

import os
import sys

for _p in ("/opt/trn_rl_repo",):
    if _p not in sys.path:
        sys.path.append(_p)

import numpy as np
import ml_dtypes

import concourse.bass as bass
import concourse.bacc as bacc
import concourse.tile as tile
from concourse import mybir

F32 = mybir.dt.float32
BF16 = mybir.dt.bfloat16
AF = mybir.ActivationFunctionType
ALU = mybir.AluOpType
AX = mybir.AxisListType

# incremental-feature flags (all algorithmic swaps default ON; structural
# PSUM re-layouts default OFF until proven hang-free on HW)
F_SQRT_LN = os.environ.get("K_SQRT_LN", "1") == "1"
F_TANH = os.environ.get("K_TANH", "1") == "1"
F_APPROX = os.environ.get("K_APPROX", "1") == "1"
F_DIRECT_OUT = os.environ.get("K_DIRECT_OUT", "1") == "1"
# tensor_tensor_reduce hangs TRN2 hardware here (sim passes) — keep off
F_TTR = os.environ.get("K_TTR", "0") == "1"
# shared multi-bank PSUM tiles + batched exp/mul (fewer, larger ops)
F_BIG = os.environ.get("K_BIG", "1") == "1"

P = 128          # partitions
D = 128          # d_pair
NH = 4           # heads
CH = 32          # head dim
HC = NH * CH     # 128


def build_nc(N=384, n_cores=8):
    C3 = N // P           # chunks along the attention axis
    R = N // n_cores      # rows per core

    nc = bacc.Bacc(
        "TRN2",
        target_bir_lowering=False,
        debug=False,
        enable_asserts=False,
        num_devices=n_cores,
    )

    Zr = nc.dram_tensor("z_raw", [R, N, D], F32, kind="ExternalInput").ap()
    Zm = nc.dram_tensor("z_mask", [R, N], F32, kind="ExternalInput").ap()
    lnw_d = nc.dram_tensor("ln_w", [D], F32, kind="ExternalInput").ap()
    lnb_d = nc.dram_tensor("ln_b", [D], F32, kind="ExternalInput").ap()
    wb_d = nc.dram_tensor("w_b", [D, NH], F32, kind="ExternalInput").ap()
    wqkv_d = nc.dram_tensor("w_qkv", [D, 3 * HC], F32, kind="ExternalInput").ap()
    wg_d = nc.dram_tensor("w_gate", [D, HC], F32, kind="ExternalInput").ap()
    gb_d = nc.dram_tensor("gating_bias", [HC], F32, kind="ExternalInput").ap()
    wo_d = nc.dram_tensor("w_o", [HC, D], F32, kind="ExternalInput").ap()
    ob_d = nc.dram_tensor("out_bias", [D], F32, kind="ExternalInput").ap()
    OUT = nc.dram_tensor("out", [R, N, D], F32, kind="ExternalOutput").ap()

    id_bf_d = nc.inline_tensor(np.eye(P, dtype=ml_dtypes.bfloat16), "id_bf_c").ap()
    id_f_d = nc.inline_tensor(np.eye(P, dtype=np.float32), "id_f_c").ap()
    sums_w = 2.0 if F_TANH else 1.0
    ones_d = nc.inline_tensor(
        np.full((P, CH), sums_w, dtype=ml_dtypes.bfloat16), "ones_c"
    ).ap()
    obb_np = np.zeros((P, P), dtype=np.float32)  # placeholder, filled on-device

    with tile.TileContext(nc) as tc:
        with (
            tc.tile_pool(name="const", bufs=1) as constp,
            tc.tile_pool(name="res", bufs=1) as resp,
            tc.tile_pool(name="work", bufs=3) as work,
            tc.tile_pool(name="stat", bufs=4) as statp,
            tc.tile_pool(name="wpool", bufs=4) as wpool,
            tc.tile_pool(name="ps", bufs=1, space="PSUM") as psum,
            tc.tile_pool(name="dram", bufs=1, space="DRAM") as dramp,
        ):
            # ---- constants / weights ----
            id_bf = constp.tile([P, P], BF16)
            nc.sync.dma_start(id_bf, id_bf_d)
            id_f = constp.tile([P, P], F32)
            nc.sync.dma_start(id_f, id_f_d)
            ones_bf = constp.tile([P, CH], BF16)
            nc.sync.dma_start(ones_bf, ones_d)

            lnw = constp.tile([D, 1], F32)
            nc.sync.dma_start(lnw, lnw_d[:, None])
            lnb = constp.tile([D, 1], F32)
            nc.sync.dma_start(lnb, lnb_d[:, None])
            ob = constp.tile([D, 1], F32)
            nc.sync.dma_start(ob, ob_d[:, None])
            gb = constp.tile([HC, 1], F32)
            nc.sync.dma_start(gb, gb_d[:, None])
            ngb = constp.tile([HC, 1], F32)
            nc.scalar.mul(ngb, gb, 0.5 if F_TANH else -1.0)
            eps_c = constp.tile([P, 1], F32)
            nc.gpsimd.memset(eps_c, 1e-5)
            neg1e9_c = constp.tile([P, 1], F32)
            nc.gpsimd.memset(neg1e9_c, -1e9)

            wtmp = constp.tile([D, 3 * HC], F32, tag="wtmp")
            nc.sync.dma_start(wtmp, wqkv_d)
            wq = constp.tile([D, HC], BF16)
            nc.scalar.activation(wq, wtmp[:, 0:HC], AF.Copy, scale=CH ** -0.5)
            wk = constp.tile([D, HC], BF16)
            nc.scalar.copy(wk, wtmp[:, HC:2 * HC])
            wv = constp.tile([D, HC], BF16)
            nc.scalar.copy(wv, wtmp[:, 2 * HC:3 * HC])

            wgt = constp.tile([D, HC], F32, tag="wgt")
            nc.sync.dma_start(wgt, wg_d)
            wg = constp.tile([D, HC], BF16)
            nc.scalar.copy(wg, wgt)
            wot = constp.tile([HC, D], F32, tag="wot")
            nc.sync.dma_start(wot, wo_d)
            wo = constp.tile([HC, D], BF16)
            nc.scalar.copy(wo, wot)
            wbt = constp.tile([D, NH], F32, tag="wbt")
            nc.sync.dma_start(wbt, wb_d)
            wb = constp.tile([D, NH], BF16)
            nc.scalar.copy(wb, wbt)

            # out_bias broadcast [tok, d] for the residual stage
            obb = constp.tile([P, P], F32)
            if F_DIRECT_OUT:
                obr = constp.tile([1, D], F32)
                nc.sync.dma_start(obr, ob_d[None, :])
                ones1 = constp.tile([1, P], F32)
                nc.gpsimd.memset(ones1, 1.0)
                obp = psum.tile(
                    [P, P], F32,
                    tag="out" if F_BIG else "logits",
                    bufs=2 if F_BIG else 3,
                    name="obp",
                )
                nc.tensor.matmul(obp, ones1, obr)
                nc.scalar.copy(obb, obp)

            # mask bias columns: mb[kc][k, i] = (Z_mask[i, k] - 1) * 1e9
            mb = []
            for kc in range(C3):
                mk = work.tile([P, R], F32, tag="mk")
                nc.sync.dma_start(
                    mk, Zm[:, kc * P:(kc + 1) * P].rearrange("r p -> p r")
                )
                mbt = resp.tile([P, R], F32, tag=f"mb{kc}", name=f"mb{kc}")
                nc.scalar.activation(mbt, mk, AF.Identity, scale=1e9, bias=neg1e9_c)
                mb.append(mbt)

            # DRAM bounce buffers for the bias AllGather
            b_shard = dramp.tile([R, NH, N], BF16, tag="bshard")
            b_full = dramp.tile(
                [n_cores * R, NH, N], BF16, tag="bfull", addr_space="Shared"
            )

            # ---- phase 1: LayerNorm -> resident Z^T, bias shard ----
            Zt = resp.tile([P, R * C3 * P], BF16, tag="Zt")
            for q in range(R):
                zrow = work.tile([P, C3, P], F32, tag="zrow")
                nc.sync.dma_start(zrow, Zr[q].rearrange("(c p) d -> p c d", p=P))
                s1 = statp.tile([P, C3], F32, tag="s1")
                nc.vector.reduce_sum(s1, zrow, axis=AX.X)
                s2 = statp.tile([P, C3], F32, tag="s2")
                if F_TTR:
                    scr = work.tile([P, P], F32, tag="scr")
                    for c in range(C3):
                        nc.vector.tensor_tensor_reduce(
                            scr, zrow[:, c, :], zrow[:, c, :],
                            scale=1.0, scalar=0.0,
                            op0=ALU.mult, op1=ALU.add,
                            accum_out=s2[:, c:c + 1],
                        )
                else:
                    zsq = work.tile([P, C3, P], F32, tag="zsq")
                    nc.scalar.square(zsq, zrow)
                    nc.vector.reduce_sum(s2, zsq, axis=AX.X)
                mu = statp.tile([P, C3], F32, tag="mu")
                nc.scalar.mul(mu, s1, 1.0 / D)
                musq = statp.tile([P, C3], F32, tag="musq")
                nc.vector.tensor_mul(musq, mu, mu)
                var = statp.tile([P, C3], F32, tag="var")
                nc.vector.scalar_tensor_tensor(
                    var, s2, 1.0 / D, musq, op0=ALU.mult, op1=ALU.subtract
                )
                rsig = statp.tile([P, C3], F32, tag="rsig")
                if F_SQRT_LN:
                    std = statp.tile([P, C3], F32, tag="std")
                    nc.scalar.activation(std, var, AF.Sqrt, bias=eps_c)
                    nc.vector.reciprocal(rsig, std)
                else:
                    lv = statp.tile([P, C3], F32, tag="lv")
                    nc.scalar.activation(lv, var, AF.Ln, bias=eps_c)
                    nc.scalar.activation(rsig, lv, AF.Exp, scale=-0.5)
                nmr = statp.tile([P, C3], F32, tag="nmr")
                nc.vector.scalar_tensor_tensor(
                    nmr, mu, -1.0, rsig, op0=ALU.mult, op1=ALU.mult
                )
                if F_BIG:
                    tp = psum.tile([P, C3, P], BF16, tag="out", bufs=2, name="tp")
                    for c in range(C3):
                        zn = work.tile([P, P], BF16, tag="zn")
                        nc.vector.tensor_scalar(
                            zn, zrow[:, c, :], rsig[:, c:c + 1], nmr[:, c:c + 1],
                            op0=ALU.mult, op1=ALU.add,
                        )
                        nc.tensor.transpose(tp[:, c, :], zn, id_bf)
                    nc.vector.tensor_scalar(
                        Zt[:, q * C3 * P:(q + 1) * C3 * P].rearrange(
                            "p (c q2) -> p c q2", c=C3
                        ),
                        tp, lnw, lnb, op0=ALU.mult, op1=ALU.add,
                    )
                else:
                    for c in range(C3):
                        zn = work.tile([P, P], BF16, tag="zn")
                        nc.vector.tensor_scalar(
                            zn, zrow[:, c, :], rsig[:, c:c + 1], nmr[:, c:c + 1],
                            op0=ALU.mult, op1=ALU.add,
                        )
                        tp = psum.tile([P, P], BF16, tag="logits", bufs=3, name="tp")
                        nc.tensor.transpose(tp, zn, id_bf)
                        nc.vector.tensor_scalar(
                            Zt[:, (q * C3 + c) * P:(q * C3 + c + 1) * P],
                            tp, lnw, lnb, op0=ALU.mult, op1=ALU.add,
                        )
                bp = psum.tile(
                    [NH, N], F32,
                    tag="acc" if F_BIG else "sum",
                    bufs=1, name="bp",
                )
                nc.tensor.matmul(bp, wb, Zt[:, q * C3 * P:(q + 1) * C3 * P])
                bsb = work.tile([NH, N], BF16, tag="bsb")
                nc.vector.tensor_copy(bsb, bp)
                nc.sync.dma_start(b_shard[q], bsb)

            nc.gpsimd.collective_compute(
                "AllGather",
                ALU.bypass,
                replica_groups=[list(range(n_cores))],
                ins=[b_shard.opt()],
                outs=[b_full.opt()],
            )

            # exp of transposed bias, resident per k-chunk: Eb[kc][k, h, q]
            Eb = [
                resp.tile([P, NH, N], BF16, tag=f"eb{kc}", name=f"eb{kc}")
                for kc in range(C3)
            ]
            for qc in range(C3):
                bt = work.tile([P, NH, N], BF16, tag="bt")
                nc.sync.dma_start(bt, b_full[qc * P:(qc + 1) * P])
                for h in range(NH):
                    for kc in range(C3):
                        tp2 = psum.tile(
                            [P, P], BF16,
                            tag="out" if F_BIG else "logits",
                            bufs=2 if F_BIG else 3,
                            name="tp2",
                        )
                        nc.tensor.transpose(
                            tp2, bt[:, h, kc * P:(kc + 1) * P], id_bf
                        )
                        nc.scalar.activation(
                            Eb[kc][:, h, qc * P:(qc + 1) * P], tp2, AF.Exp
                        )

            # ---- phase 2: per-row attention ----
            for i in range(R):
                zrow2 = work.tile([P, C3, P], F32, tag="zrow2")
                nc.sync.dma_start(zrow2, Zr[i].rearrange("(c p) d -> p c d", p=P))
                zt_row = Zt[:, i * C3 * P:(i + 1) * C3 * P]

                if F_BIG:
                    # two 2-bank tiles ping-pong: q/k in A, gate/v in B
                    pjA = psum.tile([P, 2, 512], F32, tag="bigA", bufs=1, name="pjA")
                    nc.tensor.matmul(pjA[:, 0, 0:N], wq, zt_row)
                    nc.tensor.matmul(pjA[:, 1, 0:N], wk, zt_row)
                    pjB = psum.tile([P, 2, 512], F32, tag="bigB", bufs=1, name="pjB")
                    nc.tensor.matmul(pjB[:, 0, 0:N], wg, zt_row)
                    for c in range(C3):
                        nc.tensor.matmul(
                            pjB[:, 1, c * P:(c + 1) * P],
                            zt_row[:, c * P:(c + 1) * P],
                            wv,
                        )
                    qk_sb = work.tile([P, 2, N], BF16, tag="qk_sb")
                    nc.vector.tensor_copy(qk_sb, pjA[:, 0:2, 0:N])
                    qt = qk_sb[:, 0, :]
                    kt = qk_sb[:, 1, :]
                    gp = pjB[:, 0, 0:N]
                    vsb3 = work.tile([P, C3, P], BF16, tag="vsb")
                    nc.vector.tensor_copy(
                        vsb3.rearrange("p c q2 -> p (c q2)"), pjB[:, 1, 0:N]
                    )
                else:
                    qp = psum.tile([P, N], F32, tag="proj", bufs=2, name="qp")
                    nc.tensor.matmul(qp, wq, zt_row)
                    qt = work.tile([P, N], BF16, tag="qt")
                    nc.vector.tensor_copy(qt, qp)
                    kp = psum.tile([P, N], F32, tag="proj", bufs=2, name="kp")
                    nc.tensor.matmul(kp, wk, zt_row)
                    kt = work.tile([P, N], BF16, tag="kt")
                    nc.vector.tensor_copy(kt, kp)
                    gpt = psum.tile([P, N], F32, tag="proj", bufs=2, name="gpt")
                    nc.tensor.matmul(gpt, wg, zt_row)
                    gp = gpt
                    vp = psum.tile([P, C3, P], F32, tag="proj", bufs=2, name="vp")
                    for c in range(C3):
                        nc.tensor.matmul(
                            vp[:, c, :],
                            zt_row[:, c * P:(c + 1) * P],
                            wv,
                        )
                    vsb3 = work.tile([P, C3, P], BF16, tag="vsb")
                    nc.vector.tensor_copy(vsb3, vp)

                if F_TANH:
                    th = work.tile([P, N], BF16, tag="th")
                    nc.scalar.activation(th, gp, AF.Tanh, scale=0.5, bias=ngb)
                else:
                    eg = work.tile([P, N], BF16, tag="eg")
                    nc.scalar.activation(eg, gp, AF.Exp, scale=-1.0, bias=ngb)
                    g1 = work.tile([P, N], F32, tag="g1")
                    nc.vector.tensor_scalar_add(g1, eg, 1.0)
                    gate = work.tile([P, N], F32, tag="gate")
                    nc.vector.reciprocal(gate, g1)

                if F_BIG:
                    wap3 = psum.tile([P, 2, 512], F32, tag="acc", bufs=1, name="wap3")
                    wap = wap3[:, 0, 0:N]
                    sp = wap3[:, 1, 0:N]
                else:
                    wapt = psum.tile([P, N], F32, tag="wa", bufs=1, name="wapt")
                    spt = psum.tile([P, N], F32, tag="sum", bufs=1, name="spt")
                    wap, sp = wapt, spt

                for kc in range(C3):
                    if F_BIG:
                        wms = []
                        for half, tag in ((0, "bigA"), (1, "bigB")):
                            lgH = psum.tile(
                                [P, 2, 512], F32, tag=tag, bufs=1, name=f"lg{tag}"
                            )
                            for hh in range(2):
                                h = half * 2 + hh
                                nc.tensor.matmul(
                                    lgH[:, hh, 0:N],
                                    kt[CH * h:CH * (h + 1), kc * P:(kc + 1) * P],
                                    qt[CH * h:CH * (h + 1), :],
                                    tile_position=(CH * h, 0),
                                )
                            wH = wpool.tile([P, 2, N], BF16, tag=f"wt{half}")
                            nc.scalar.activation(
                                wH, lgH[:, :, 0:N], AF.Exp, bias=mb[kc][:, i:i + 1]
                            )
                            wmH = wpool.tile([P, 2, N], BF16, tag=f"wm{half}")
                            nc.vector.tensor_mul(
                                wmH, wH, Eb[kc][:, 2 * half:2 * half + 2, :]
                            )
                            wms.extend([wmH[:, 0, :], wmH[:, 1, :]])
                    else:
                        wms = []
                        for h in range(NH):
                            lg = psum.tile([P, N], F32, tag="logits", bufs=3, name="lg")
                            nc.tensor.matmul(
                                lg,
                                kt[CH * h:CH * (h + 1), kc * P:(kc + 1) * P],
                                qt[CH * h:CH * (h + 1), :],
                                tile_position=(CH * h, 0),
                            )
                            w_t = wpool.tile([P, N], BF16, tag="wt")
                            nc.scalar.activation(
                                w_t, lg, AF.Exp, bias=mb[kc][:, i:i + 1]
                            )
                            wm = wpool.tile([P, N], BF16, tag="wm")
                            nc.vector.tensor_mul(wm, w_t, Eb[kc][:, h, :])
                            wms.append(wm)
                    for h in range(NH):
                        nc.tensor.matmul(
                            wap[CH * h:CH * (h + 1), :],
                            vsb3[:, kc, CH * h:CH * (h + 1)],
                            wms[h],
                            start=(kc == 0),
                            stop=(kc == C3 - 1),
                            skip_group_check=True,
                            tile_position=(0, CH * h),
                        )
                    for h in range(NH):
                        nc.tensor.matmul(
                            sp[CH * h:CH * (h + 1), :],
                            ones_bf,
                            wms[h],
                            start=(kc == 0),
                            stop=(kc == C3 - 1),
                            skip_group_check=True,
                            tile_position=(0, CH * h),
                        )

                rs = work.tile([P, N], F32, tag="rs")
                if F_APPROX:
                    nc.vector.reciprocal_approx_fast(rs, sp)
                else:
                    nc.vector.reciprocal(rs, sp)
                wan = work.tile([P, N], F32, tag="wan")
                nc.vector.tensor_mul(wan, wap, rs)
                gwa = work.tile([P, N], BF16, tag="gwa")
                if F_TANH:
                    # gwa = (tanh+1) * (wa / 2s) == sigmoid * wa / s
                    nc.vector.scalar_tensor_tensor(
                        gwa, th, 1.0, wan, op0=ALU.add, op1=ALU.mult
                    )
                else:
                    nc.vector.tensor_mul(gwa, wan, gate)

                if F_DIRECT_OUT:
                    out_ps = psum.tile(
                        [P, C3, P], F32,
                        tag="out" if F_BIG else "oproj",
                        bufs=2 if F_BIG else 1,
                        name="out_ps",
                    )
                    for c in range(C3):
                        nc.tensor.matmul(
                            out_ps[:, c, :], gwa[:, c * P:(c + 1) * P], wo
                        )
                    fin = work.tile([P, C3, P], F32, tag="fin")
                    nc.vector.tensor_add(fin, out_ps, zrow2)
                    for c in range(C3):
                        nc.vector.tensor_add(fin[:, c, :], fin[:, c, :], obb)
                else:
                    op_ = psum.tile([P, N], F32, tag="oproj", bufs=1, name="op_")
                    nc.tensor.matmul(op_, wo, gwa)
                    osb = work.tile([P, N], F32, tag="osb")
                    nc.scalar.activation(osb, op_, AF.Identity, bias=ob)
                    ot = psum.tile([P, C3, P], F32, tag="proj", bufs=2, name="ot")
                    for c in range(C3):
                        nc.tensor.transpose(
                            ot[:, c, :], osb[:, c * P:(c + 1) * P], id_f
                        )
                    fin = work.tile([P, C3, P], F32, tag="fin")
                    nc.vector.tensor_add(fin, ot, zrow2)
                nc.sync.dma_start(OUT[i].rearrange("(c p) d -> p c d", p=P), fin)

    nc.compile()
    return nc


_CACHE = {}


def get_nc(N=384, n_cores=8):
    key = (N, n_cores)
    if key not in _CACHE:
        _CACHE[key] = build_nc(N, n_cores)
    return _CACHE[key]


def make_in_maps(inputs, N=384, n_cores=8):
    R = N // n_cores
    Z = np.ascontiguousarray(np.asarray(inputs["Z_raw"], dtype=np.float32))
    M = np.ascontiguousarray(np.asarray(inputs["Z_mask"], dtype=np.float32))
    shared = {
        "ln_w": np.ascontiguousarray(np.asarray(inputs["ln_w"], np.float32)),
        "ln_b": np.ascontiguousarray(np.asarray(inputs["ln_b"], np.float32)),
        "w_b": np.ascontiguousarray(np.asarray(inputs["W_b"], np.float32)),
        "w_qkv": np.ascontiguousarray(np.asarray(inputs["W_qkv"], np.float32)),
        "w_gate": np.ascontiguousarray(np.asarray(inputs["W_gate"], np.float32)),
        "gating_bias": np.ascontiguousarray(
            np.asarray(inputs["gating_bias"], np.float32)
        ),
        "w_o": np.ascontiguousarray(np.asarray(inputs["W_o"], np.float32)),
        "out_bias": np.ascontiguousarray(np.asarray(inputs["out_bias"], np.float32)),
    }
    in_maps = []
    for c in range(n_cores):
        m = dict(shared)
        m["z_raw"] = np.ascontiguousarray(Z[0, c * R:(c + 1) * R])
        m["z_mask"] = np.ascontiguousarray(M[0, c * R:(c + 1) * R])
        in_maps.append(m)
    return in_maps


def kernel(**inputs):
    from concourse.bass_utils import run_bass_kernel_spmd

    N, n_cores = 384, 8
    nc = get_nc(N, n_cores)
    in_maps = make_in_maps(inputs, N, n_cores)
    res = run_bass_kernel_spmd(nc, in_maps, list(range(n_cores)))
    out = np.concatenate([res.results[c]["out"] for c in range(n_cores)], axis=0)
    return out.reshape(1, N, N, D).astype(np.float32)
